# revision 18
# baseline (speedup 1.0000x reference)
"""MDN-RNN (LSTM + MDN head) Trainium2 kernel, data-parallel over batch on 8 cores.

Shapes: B=128, T=512, Z=32, A=3 -> IN=35, H=256, K=5, MDN=325.
Per-core batch shard Bl=16.

Layout notes (per core):
  - Gate tile order (after host col-permutation of Wx/Wh/b): [i0,i1,f0,f1,o0,o1,g0,g1]
    so gates PSUM (128 part = within-tile gate dim, 8 groups x 16 batch cols):
    i = cols 0:32, f = 32:64, o = 64:96, g = 96:128.
  - xT (35, Bl*T) fp32, token = t*16 + b (step-major).
  - xprojT chunk buffer (128, Tc*128) fp32: col = tl*128 + m*16 + b.
  - hT history (128, 2*(T+1)*16) bf16: col = k*(T+1)*16 + (t+1)*16 + b; slot 0 = h_{-1}=0.
  - head: out (tokens, 325) = hT_tile.T @ Wd (both bf16), tokens on partitions.
"""
import sys
import os

sys.path.insert(0, "/opt/trn_rl_repo")

import numpy as np
import ml_dtypes

import concourse.bass as bass
import concourse.mybir as mybir
import concourse.tile as tile
from concourse import bacc

F32 = mybir.dt.float32
F32R = mybir.dt.float32r
BF16 = mybir.dt.bfloat16
AF = mybir.ActivationFunctionType

P = 128
B_FULL, T, Z, A = 128, 512, 32, 3
IN = Z + A            # 35
H = 256               # hidden
G4 = 4 * H            # 1024
K_MIX = 5
MDN = K_MIX * (1 + 2 * Z)  # 325
NCORE = 8
BL = B_FULL // NCORE  # 16
TC = 64               # chunk length in steps
NCHUNK = T // TC
TOK = BL * T          # 8192 tokens per core
SLOTS = T + 1         # h history slots
HCOL = SLOTS * BL     # columns per k-half in hT

# gate m-tile order: which 128-col slice of the original [i f g o] weight matrix
# lands in psum col-group j. groups: i=0:2, f=2:4, o=4:6, g=6:8
GATE_PERM = [0, 1, 2, 3, 6, 7, 4, 5]


def build(t_steps=T):
    nchunk = t_steps // TC
    nc = bacc.Bacc(None, target_bir_lowering=False)

    xT_d = nc.dram_tensor("xT", [IN, BL * t_steps], BF16, kind="ExternalInput")
    wh_d = nc.dram_tensor("wh", [P, 2 * G4], BF16, kind="ExternalInput")
    wx_d = nc.dram_tensor("wx", [IN, G4], BF16, kind="ExternalInput")
    bvec_d = nc.dram_tensor("bvec", [P, 8], F32, kind="ExternalInput")
    wd_d = nc.dram_tensor("wd", [P, 2 * MDN], BF16, kind="ExternalInput")
    bdb_d = nc.dram_tensor("bdb", [P, MDN], F32, kind="ExternalInput")
    id_d = nc.dram_tensor("ident", [P, P], BF16, kind="ExternalInput")

    logpi_d = nc.dram_tensor("logpi", [BL * t_steps, K_MIX], F32, kind="ExternalOutput")
    mu_d = nc.dram_tensor("mu", [BL * t_steps, K_MIX * Z], F32, kind="ExternalOutput")
    sigma_d = nc.dram_tensor("sigma", [BL * t_steps, K_MIX * Z], F32, kind="ExternalOutput")
    cout_d = nc.dram_tensor("cout", [BL, H], F32, kind="ExternalOutput")
    hout_d = nc.dram_tensor("hout", [BL, H], F32, kind="ExternalOutput")

    hcol = (t_steps + 1) * BL
    ntile_tok = (BL * t_steps) // P  # head token tiles total

    with tile.TileContext(nc) as tc:
        with (
            tc.tile_pool(name="const", bufs=1) as cpool,
            tc.tile_pool(name="state", bufs=1) as spool,
            tc.tile_pool(name="xproj", bufs=2) as xppool,
            tc.tile_pool(name="acts", bufs=3) as apool,
            tc.tile_pool(name="tmp", bufs=3) as tpool,
            tc.tile_pool(name="mdnsb", bufs=3) as mpool,
            tc.tile_pool(name="stage", bufs=1) as stpool,
            tc.tile_pool(name="ps_gates", bufs=2, space="PSUM") as gates_ps,
            tc.tile_pool(name="ps_xp", bufs=2, space="PSUM") as xp_ps,
            tc.tile_pool(name="ps_mdn", bufs=2, space="PSUM") as mdn_ps,
        ):
            # ---- constants / inputs in SBUF ----
            xT = cpool.tile([IN, BL * t_steps], BF16)
            wh = cpool.tile([P, 2 * G4], BF16)
            wx = cpool.tile([IN, G4], BF16)
            bvec = cpool.tile([P, 8], F32)
            wd = cpool.tile([P, 2 * MDN], BF16)
            bdb = cpool.tile([P, MDN], F32)
            ident = cpool.tile([P, P], BF16)
            nc.sync.dma_start(xT[:], xT_d[:])
            nc.sync.dma_start(wh[:], wh_d[:])
            nc.sync.dma_start(wx[:], wx_d[:])
            nc.sync.dma_start(bvec[:], bvec_d[:])
            nc.sync.dma_start(wd[:], wd_d[:])
            nc.sync.dma_start(bdb[:], bdb_d[:])
            nc.sync.dma_start(ident[:], id_d[:])

            # ---- state ----
            hT = spool.tile([P, 2 * hcol], BF16)   # h history, bf16
            cT = spool.tile([P, 2 * BL], F32)      # cell state
            hfin = spool.tile([P, 2 * BL], F32)    # final-step h in fp32
            pi_sb = stpool.tile([P, ntile_tok * K_MIX], F32)
            ls_sb = stpool.tile([P, ntile_tok * K_MIX * Z], F32)

            nc.vector.memset(cT[:], 0.0)
            # zero h_{-1} slots (slot 0 of each k half)
            nc.vector.memset(hT[:, 0:BL], 0.0)
            nc.vector.memset(hT[:, hcol:hcol + BL], 0.0)

            for c in range(nchunk):
                # ---------- xproj phase: chunk c ----------
                xp = xppool.tile([P, TC * P], BF16)  # (128, 8192)
                xp4 = xp[:].rearrange("p (tl m b) -> p tl m b", m=8, b=BL)
                for m in range(8):
                    for nn in range(TC * BL // 512):
                        ps = xp_ps.tile([P, 512], F32)
                        nc.tensor.matmul(
                            ps[:],
                            lhsT=wx[:, m * P:(m + 1) * P],
                            rhs=xT[:, c * TC * BL + nn * 512: c * TC * BL + (nn + 1) * 512],
                            start=True, stop=True,
                        )
                        ps3 = ps[:].rearrange("p (tl b) -> p tl b", b=BL)
                        dest = xp4[:, nn * 32:(nn + 1) * 32, m, :]
                        if nn == 0:
                            nc.vector.tensor_scalar_add(dest, ps3, bvec[:, m:m + 1])
                        else:
                            nc.scalar.activation(dest, ps3, AF.Identity, bias=bvec[:, m:m + 1])

                # ---------- recurrence: chunk c ----------
                for tl in range(TC):
                    t = c * TC + tl
                    psA = gates_ps.tile([P, 96], F32, tag="psA")
                    psB = gates_ps.tile([P, 32], F32, tag="psB")
                    nc.tensor.matmul(
                        psA[:], lhsT=ident[:], rhs=xp[:, tl * P: tl * P + 96],
                        start=True, stop=False,
                    )
                    nc.tensor.matmul(
                        psB[:], lhsT=ident[:], rhs=xp[:, tl * P + 96: (tl + 1) * P],
                        start=True, stop=False,
                    )
                    for k in range(2):
                        for m in range(6):
                            nc.tensor.matmul(
                                psA[:, m * BL:(m + 1) * BL],
                                lhsT=wh[:, k * G4 + m * P: k * G4 + (m + 1) * P],
                                rhs=hT[:, k * hcol + t * BL: k * hcol + (t + 1) * BL],
                                start=False, stop=(m == 5 and k == 1),
                            )
                        for m in range(6, 8):
                            nc.tensor.matmul(
                                psB[:, (m - 6) * BL:(m - 5) * BL],
                                lhsT=wh[:, k * G4 + m * P: k * G4 + (m + 1) * P],
                                rhs=hT[:, k * hcol + t * BL: k * hcol + (t + 1) * BL],
                                start=False, stop=(m == 7 and k == 1),
                            )
                    act = apool.tile([P, P], F32)
                    nc.scalar.activation(act[:, 0:96], psA[:], AF.Sigmoid)
                    nc.scalar.activation(act[:, 96:128], psB[:], AF.Tanh)
                    ig = tpool.tile([P, 2 * BL], F32, tag="ig")
                    fc = tpool.tile([P, 2 * BL], F32, tag="fc")
                    th = tpool.tile([P, 2 * BL], F32, tag="th")
                    nc.vector.tensor_mul(ig[:], act[:, 0:32], act[:, 96:128])
                    nc.vector.tensor_mul(fc[:], act[:, 32:64], cT[:])
                    nc.vector.tensor_add(cT[:], ig[:], fc[:])
                    nc.scalar.activation(th[:], cT[:], AF.Tanh)
                    nc.vector.tensor_mul(
                        hT[:, 0 * hcol + (t + 1) * BL: 0 * hcol + (t + 2) * BL],
                        act[:, 64:80], th[:, 0:16],
                    )
                    nc.vector.tensor_mul(
                        hT[:, 1 * hcol + (t + 1) * BL: 1 * hcol + (t + 2) * BL],
                        act[:, 80:96], th[:, 16:32],
                    )
                    if t == t_steps - 1:
                        nc.vector.tensor_mul(hfin[:], act[:, 64:96], th[:])

                # ---------- MDN head: chunk c ----------
                for g in range(TC // 8):
                    gi = c * (TC // 8) + g
                    t0 = c * TC + g * 8
                    ps = mdn_ps.tile([P, MDN], F32)
                    for k in range(2):
                        nc.tensor.matmul(
                            ps[:],
                            lhsT=hT[:, k * hcol + (t0 + 1) * BL: k * hcol + (t0 + 9) * BL],
                            rhs=wd[:, k * MDN:(k + 1) * MDN],
                            start=(k == 0), stop=(k == 1),
                        )
                    sb = mpool.tile([P, MDN], F32)
                    nc.vector.tensor_add(sb[:], ps[:], bdb[:])
                    # mu straight out
                    nc.sync.dma_start(
                        mu_d[gi * P:(gi + 1) * P, :],
                        sb[:, K_MIX:K_MIX + K_MIX * Z],
                    )
                    # stage pi and log_sigma
                    nc.vector.tensor_copy(pi_sb[:, gi * K_MIX:(gi + 1) * K_MIX], sb[:, 0:K_MIX])
                    nc.vector.tensor_copy(
                        ls_sb[:, gi * K_MIX * Z:(gi + 1) * K_MIX * Z],
                        sb[:, K_MIX + K_MIX * Z:],
                    )

            # ---------- postproc ----------
            # sigma = exp(ls) + 1e-6
            nc.scalar.activation(ls_sb[:], ls_sb[:], AF.Exp)
            nc.vector.tensor_scalar_add(ls_sb[:], ls_sb[:], 1e-6)
            # log_pi = pi - ln(sum(exp(pi), axis=group))
            expv = stpool.tile([P, ntile_tok * K_MIX], F32)
            sums = stpool.tile([P, ntile_tok], F32)
            lns = stpool.tile([P, ntile_tok], F32)
            nc.scalar.activation(expv[:], pi_sb[:], AF.Exp)
            nc.vector.tensor_reduce(
                sums[:].unsqueeze(2),
                expv[:].rearrange("p (g j) -> p g j", j=K_MIX),
                axis=mybir.AxisListType.X, op=mybir.AluOpType.add,
            )
            nc.scalar.activation(lns[:], sums[:], AF.Ln)
            lpi = stpool.tile([P, ntile_tok * K_MIX], F32)
            nc.vector.tensor_tensor(
                lpi[:].rearrange("p (g j) -> p g j", j=K_MIX),
                pi_sb[:].rearrange("p (g j) -> p g j", j=K_MIX),
                lns[:].unsqueeze(2).to_broadcast([P, ntile_tok, K_MIX]),
                mybir.AluOpType.subtract,
            )
            # DMA outs per token tile
            for gi in range(ntile_tok):
                t0 = gi * 8
                nc.sync.dma_start(
                    logpi_d[gi * P:(gi + 1) * P, :],
                    lpi[:, gi * K_MIX:(gi + 1) * K_MIX],
                )
                nc.sync.dma_start(
                    sigma_d[gi * P:(gi + 1) * P, :],
                    ls_sb[:, gi * K_MIX * Z:(gi + 1) * K_MIX * Z],
                )

            # final carry
            for k in range(2):
                nc.sync.dma_start(
                    cout_d[:, k * P:(k + 1) * P].transpose([1, 0]),
                    cT[:, k * BL:(k + 1) * BL],
                )
                nc.sync.dma_start(
                    hout_d[:, k * P:(k + 1) * P].transpose([1, 0]),
                    hfin[:, k * BL:(k + 1) * BL],
                )

    nc.finalize()
    return nc


# ---------------- host side ----------------
_cached = {}


def _get_nc(t_steps):
    if t_steps not in _cached:
        _cached[t_steps] = build(t_steps)
    return _cached[t_steps]


def prep_inputs(z_t, a_t, Wx, Wh, b, Wd, bd, t_steps=T):
    """Build per-core input maps."""
    z_t = np.asarray(z_t, np.float32)[:, :t_steps]
    a_t = np.asarray(a_t, np.float32)[:, :t_steps]
    Wx = np.asarray(Wx, np.float32)
    Wh = np.asarray(Wh, np.float32)
    b = np.asarray(b, np.float32)
    Wd = np.asarray(Wd, np.float32)
    bd = np.asarray(bd, np.float32)

    perm_cols = np.concatenate([np.arange(j * P, (j + 1) * P) for j in GATE_PERM])
    Wh_p = Wh[:, perm_cols]                     # (256, 1024)
    Wx_p = Wx[:, perm_cols]                     # (35, 1024)
    b_p = b[perm_cols]                          # (1024,)

    wh_host = np.ascontiguousarray(
        Wh_p.reshape(2, P, G4).transpose(1, 0, 2).reshape(P, 2 * G4)
    ).astype(ml_dtypes.bfloat16)
    # wh layout: [p, k*G4 + col] = Wh_p[k*128 + p, col]
    bvec_host = np.ascontiguousarray(b_p.reshape(8, P).T)  # (128, 8)
    wd_host = np.ascontiguousarray(
        Wd.reshape(2, P, MDN).transpose(1, 0, 2).reshape(P, 2 * MDN)
    ).astype(ml_dtypes.bfloat16)
    bdb_host = np.broadcast_to(bd, (P, MDN)).copy()
    ident_host = np.eye(P, dtype=np.float32).astype(ml_dtypes.bfloat16)

    x = np.concatenate([z_t, a_t], axis=-1)     # (B, t, 35)
    in_maps = []
    for core in range(NCORE):
        xs = x[core * BL:(core + 1) * BL]       # (16, t, 35)
        xTh = np.ascontiguousarray(xs.transpose(2, 1, 0).reshape(IN, t_steps * BL))
        in_maps.append({
            "xT": xTh.astype(ml_dtypes.bfloat16),
            "wh": wh_host,
            "wx": np.ascontiguousarray(Wx_p).astype(ml_dtypes.bfloat16),
            "bvec": bvec_host,
            "wd": wd_host,
            "bdb": bdb_host,
            "ident": ident_host,
        })
    return in_maps


def run(inputs, t_steps=T, trace=False):
    from concourse.bass_utils import run_bass_kernel_spmd
    nc = _get_nc(t_steps)
    in_maps = prep_inputs(**inputs, t_steps=t_steps)
    res = run_bass_kernel_spmd(nc, in_maps, core_ids=list(range(NCORE)), trace=trace)
    return res


def _unscramble(arr, t_steps):
    # device rows are token = t*BL + b; -> (BL, t_steps, d)
    d = arr.shape[-1]
    return np.ascontiguousarray(arr.reshape(t_steps, BL, d).transpose(1, 0, 2))


def assemble(results, t_steps=T):
    logpi = np.concatenate([_unscramble(r["logpi"], t_steps) for r in results], axis=0)
    mu = np.concatenate([_unscramble(r["mu"], t_steps) for r in results], axis=0)
    sigma = np.concatenate([_unscramble(r["sigma"], t_steps) for r in results], axis=0)
    c = np.concatenate([r["cout"] for r in results], axis=0)
    h = np.concatenate([r["hout"] for r in results], axis=0)
    B = logpi.shape[0]
    mu = mu.reshape(B, t_steps, K_MIX, Z)
    sigma = sigma.reshape(B, t_steps, K_MIX, Z)
    return logpi, mu, sigma, (c, h)


def kernel(z_t, a_t, Wx, Wh, b, Wd, bd):
    res = run(dict(z_t=z_t, a_t=a_t, Wx=Wx, Wh=Wh, b=b, Wd=Wd, bd=bd), t_steps=T)
    return assemble(res.results, t_steps=T)


# revision 19
# speedup vs baseline: 1.0156x; 1.0156x over previous
"""MDN-RNN (LSTM + MDN head) Trainium2 kernel, data-parallel over batch on 8 cores.

Shapes: B=128, T=512, Z=32, A=3 -> IN=35, H=256, K=5, MDN=325.
Per-core batch shard Bl=16.

Layout notes (per core):
  - Gate tile order (after host col-permutation of Wx/Wh/b): [i0,i1,f0,f1,o0,o1,g0,g1]
    so gates PSUM (128 part = within-tile gate dim, 8 groups x 16 batch cols):
    i = cols 0:32, f = 32:64, o = 64:96, g = 96:128.
  - xT (35, Bl*T) fp32, token = t*16 + b (step-major).
  - xprojT chunk buffer (128, Tc*128) fp32: col = tl*128 + m*16 + b.
  - hT history (128, 2*(T+1)*16) bf16: col = k*(T+1)*16 + (t+1)*16 + b; slot 0 = h_{-1}=0.
  - head: out (tokens, 325) = hT_tile.T @ Wd (both bf16), tokens on partitions.
"""
import sys
import os

sys.path.insert(0, "/opt/trn_rl_repo")

import numpy as np
import ml_dtypes

import concourse.bass as bass
import concourse.mybir as mybir
import concourse.tile as tile
from concourse import bacc

F32 = mybir.dt.float32
F32R = mybir.dt.float32r
BF16 = mybir.dt.bfloat16
AF = mybir.ActivationFunctionType

P = 128
B_FULL, T, Z, A = 128, 512, 32, 3
IN = Z + A            # 35
H = 256               # hidden
G4 = 4 * H            # 1024
K_MIX = 5
MDN = K_MIX * (1 + 2 * Z)  # 325
NCORE = 8
BL = B_FULL // NCORE  # 16
TC = 64               # chunk length in steps
NCHUNK = T // TC
TOK = BL * T          # 8192 tokens per core
SLOTS = T + 1         # h history slots
HCOL = SLOTS * BL     # columns per k-half in hT

# gate m-tile order: which 128-col slice of the original [i f g o] weight matrix
# lands in psum col-group j. groups: i=0:2, f=2:4, o=4:6, g=6:8
GATE_PERM = [0, 1, 2, 3, 6, 7, 4, 5]


def build(t_steps=T):
    nchunk = t_steps // TC
    nc = bacc.Bacc(None, target_bir_lowering=False)

    xT_d = nc.dram_tensor("xT", [IN, BL * t_steps], BF16, kind="ExternalInput")
    wh_d = nc.dram_tensor("wh", [P, 2 * G4], BF16, kind="ExternalInput")
    wx_d = nc.dram_tensor("wx", [IN, G4], BF16, kind="ExternalInput")
    bvec_d = nc.dram_tensor("bvec", [P, 8], F32, kind="ExternalInput")
    wd_d = nc.dram_tensor("wd", [P, 2 * MDN], BF16, kind="ExternalInput")
    bdb_d = nc.dram_tensor("bdb", [P, MDN], F32, kind="ExternalInput")
    id_d = nc.dram_tensor("ident", [P, P], BF16, kind="ExternalInput")

    logpi_d = nc.dram_tensor("logpi", [BL * t_steps, K_MIX], F32, kind="ExternalOutput")
    mu_d = nc.dram_tensor("mu", [BL * t_steps, K_MIX * Z], F32, kind="ExternalOutput")
    sigma_d = nc.dram_tensor("sigma", [BL * t_steps, K_MIX * Z], F32, kind="ExternalOutput")
    cout_d = nc.dram_tensor("cout", [BL, H], F32, kind="ExternalOutput")
    hout_d = nc.dram_tensor("hout", [BL, H], F32, kind="ExternalOutput")

    hcol = (t_steps + 1) * BL
    ntile_tok = (BL * t_steps) // P  # head token tiles total

    with tile.TileContext(nc) as tc:
        with (
            tc.tile_pool(name="const", bufs=1) as cpool,
            tc.tile_pool(name="state", bufs=1) as spool,
            tc.tile_pool(name="xproj", bufs=2) as xppool,
            tc.tile_pool(name="acts", bufs=3) as apool,
            tc.tile_pool(name="tmp", bufs=3) as tpool,
            tc.tile_pool(name="mdnsb", bufs=3) as mpool,
            tc.tile_pool(name="stage", bufs=1) as stpool,
            tc.tile_pool(name="ps_gates", bufs=2, space="PSUM") as gates_ps,
            tc.tile_pool(name="ps_xp", bufs=2, space="PSUM") as xp_ps,
            tc.tile_pool(name="ps_mdn", bufs=2, space="PSUM") as mdn_ps,
        ):
            # ---- constants / inputs in SBUF ----
            xT = cpool.tile([IN, BL * t_steps], BF16)
            wh = cpool.tile([P, 2 * G4], BF16)
            wx = cpool.tile([IN, G4], BF16)
            bvec = cpool.tile([P, 8], F32)
            wd = cpool.tile([P, 2 * MDN], BF16)
            bdb = cpool.tile([P, MDN], F32)
            ident = cpool.tile([P, P], BF16)
            nc.sync.dma_start(xT[:], xT_d[:])
            nc.sync.dma_start(wh[:], wh_d[:])
            nc.sync.dma_start(wx[:], wx_d[:])
            nc.sync.dma_start(bvec[:], bvec_d[:])
            nc.sync.dma_start(wd[:], wd_d[:])
            nc.sync.dma_start(bdb[:], bdb_d[:])
            nc.sync.dma_start(ident[:], id_d[:])

            # ---- state ----
            hT = spool.tile([P, 2 * hcol], BF16)   # h history, bf16
            cT = spool.tile([P, 2 * BL], F32)      # cell state
            hfin = spool.tile([P, 2 * BL], F32)    # final-step h in fp32
            pi_sb = stpool.tile([P, ntile_tok * K_MIX], F32)
            ls_sb = stpool.tile([P, ntile_tok * K_MIX * Z], F32)

            nc.vector.memset(cT[:], 0.0)
            # zero h_{-1} slots (slot 0 of each k half)
            nc.vector.memset(hT[:, 0:BL], 0.0)
            nc.vector.memset(hT[:, hcol:hcol + BL], 0.0)

            for c in range(nchunk):
                # ---------- xproj phase: chunk c ----------
                xp = xppool.tile([P, TC * P], BF16)  # (128, 8192)
                xp4 = xp[:].rearrange("p (tl m b) -> p tl m b", m=8, b=BL)
                for m in range(8):
                    for nn in range(TC * BL // 512):
                        ps = xp_ps.tile([P, 512], F32)
                        nc.tensor.matmul(
                            ps[:],
                            lhsT=wx[:, m * P:(m + 1) * P],
                            rhs=xT[:, c * TC * BL + nn * 512: c * TC * BL + (nn + 1) * 512],
                            start=True, stop=True,
                        )
                        ps3 = ps[:].rearrange("p (tl b) -> p tl b", b=BL)
                        dest = xp4[:, nn * 32:(nn + 1) * 32, m, :]
                        if nn == 0:
                            nc.vector.tensor_scalar_add(dest, ps3, bvec[:, m:m + 1])
                        else:
                            nc.scalar.activation(dest, ps3, AF.Identity, bias=bvec[:, m:m + 1])

                # ---------- recurrence: chunk c ----------
                for tl in range(TC):
                    t = c * TC + tl
                    psA = gates_ps.tile([P, 96], F32, tag="psA")
                    psB = gates_ps.tile([P, 32], F32, tag="psB")
                    nc.tensor.matmul(
                        psA[:], lhsT=ident[:], rhs=xp[:, tl * P: tl * P + 96],
                        start=True, stop=False,
                    )
                    for m in range(6):
                        for k in range(2):
                            nc.tensor.matmul(
                                psA[:, m * BL:(m + 1) * BL],
                                lhsT=wh[:, k * G4 + m * P: k * G4 + (m + 1) * P],
                                rhs=hT[:, k * hcol + t * BL: k * hcol + (t + 1) * BL],
                                start=False, stop=(m == 5 and k == 1),
                            )
                    nc.tensor.matmul(
                        psB[:], lhsT=ident[:], rhs=xp[:, tl * P + 96: (tl + 1) * P],
                        start=True, stop=False,
                    )
                    for m in range(6, 8):
                        for k in range(2):
                            nc.tensor.matmul(
                                psB[:, (m - 6) * BL:(m - 5) * BL],
                                lhsT=wh[:, k * G4 + m * P: k * G4 + (m + 1) * P],
                                rhs=hT[:, k * hcol + t * BL: k * hcol + (t + 1) * BL],
                                start=False, stop=(m == 7 and k == 1),
                            )
                    act = apool.tile([P, P], F32)
                    nc.scalar.activation(act[:, 0:96], psA[:], AF.Sigmoid)
                    nc.scalar.activation(act[:, 96:128], psB[:], AF.Tanh)
                    ig = tpool.tile([P, 2 * BL], F32, tag="ig")
                    fc = tpool.tile([P, 2 * BL], F32, tag="fc")
                    th = tpool.tile([P, 2 * BL], F32, tag="th")
                    nc.vector.tensor_mul(ig[:], act[:, 0:32], act[:, 96:128])
                    nc.vector.tensor_mul(fc[:], act[:, 32:64], cT[:])
                    nc.vector.tensor_add(cT[:], ig[:], fc[:])
                    nc.scalar.activation(th[:], cT[:], AF.Tanh)
                    hdst = hT[:].rearrange("p (k s b) -> p k s b", k=2, b=BL)[:, :, t + 1, :]
                    o2 = act[:, 64:96].rearrange("p (k b) -> p k b", k=2)
                    th2 = th[:].rearrange("p (k b) -> p k b", k=2)
                    nc.vector.tensor_mul(hdst, o2, th2)
                    if t == t_steps - 1:
                        nc.vector.tensor_mul(hfin[:], act[:, 64:96], th[:])

                # ---------- MDN head: chunk c ----------
                for g in range(TC // 8):
                    gi = c * (TC // 8) + g
                    t0 = c * TC + g * 8
                    ps = mdn_ps.tile([P, MDN], F32)
                    for k in range(2):
                        nc.tensor.matmul(
                            ps[:],
                            lhsT=hT[:, k * hcol + (t0 + 1) * BL: k * hcol + (t0 + 9) * BL],
                            rhs=wd[:, k * MDN:(k + 1) * MDN],
                            start=(k == 0), stop=(k == 1),
                        )
                    sb = mpool.tile([P, MDN], F32)
                    nc.vector.tensor_add(sb[:], ps[:], bdb[:])
                    # mu straight out
                    nc.sync.dma_start(
                        mu_d[gi * P:(gi + 1) * P, :],
                        sb[:, K_MIX:K_MIX + K_MIX * Z],
                    )
                    # stage pi and log_sigma
                    nc.vector.tensor_copy(pi_sb[:, gi * K_MIX:(gi + 1) * K_MIX], sb[:, 0:K_MIX])
                    nc.vector.tensor_copy(
                        ls_sb[:, gi * K_MIX * Z:(gi + 1) * K_MIX * Z],
                        sb[:, K_MIX + K_MIX * Z:],
                    )

            # ---------- postproc ----------
            # sigma = exp(ls) + 1e-6
            nc.scalar.activation(ls_sb[:], ls_sb[:], AF.Exp)
            nc.vector.tensor_scalar_add(ls_sb[:], ls_sb[:], 1e-6)
            # log_pi = pi - ln(sum(exp(pi), axis=group))
            expv = stpool.tile([P, ntile_tok * K_MIX], F32)
            sums = stpool.tile([P, ntile_tok], F32)
            lns = stpool.tile([P, ntile_tok], F32)
            nc.scalar.activation(expv[:], pi_sb[:], AF.Exp)
            nc.vector.tensor_reduce(
                sums[:].unsqueeze(2),
                expv[:].rearrange("p (g j) -> p g j", j=K_MIX),
                axis=mybir.AxisListType.X, op=mybir.AluOpType.add,
            )
            nc.scalar.activation(lns[:], sums[:], AF.Ln)
            lpi = stpool.tile([P, ntile_tok * K_MIX], F32)
            nc.vector.tensor_tensor(
                lpi[:].rearrange("p (g j) -> p g j", j=K_MIX),
                pi_sb[:].rearrange("p (g j) -> p g j", j=K_MIX),
                lns[:].unsqueeze(2).to_broadcast([P, ntile_tok, K_MIX]),
                mybir.AluOpType.subtract,
            )
            # DMA outs per token tile
            for gi in range(ntile_tok):
                t0 = gi * 8
                nc.sync.dma_start(
                    logpi_d[gi * P:(gi + 1) * P, :],
                    lpi[:, gi * K_MIX:(gi + 1) * K_MIX],
                )
                nc.sync.dma_start(
                    sigma_d[gi * P:(gi + 1) * P, :],
                    ls_sb[:, gi * K_MIX * Z:(gi + 1) * K_MIX * Z],
                )

            # final carry
            for k in range(2):
                nc.sync.dma_start(
                    cout_d[:, k * P:(k + 1) * P].transpose([1, 0]),
                    cT[:, k * BL:(k + 1) * BL],
                )
                nc.sync.dma_start(
                    hout_d[:, k * P:(k + 1) * P].transpose([1, 0]),
                    hfin[:, k * BL:(k + 1) * BL],
                )

    nc.finalize()
    return nc


# ---------------- host side ----------------
_cached = {}


def _get_nc(t_steps):
    if t_steps not in _cached:
        _cached[t_steps] = build(t_steps)
    return _cached[t_steps]


def prep_inputs(z_t, a_t, Wx, Wh, b, Wd, bd, t_steps=T):
    """Build per-core input maps."""
    z_t = np.asarray(z_t, np.float32)[:, :t_steps]
    a_t = np.asarray(a_t, np.float32)[:, :t_steps]
    Wx = np.asarray(Wx, np.float32)
    Wh = np.asarray(Wh, np.float32)
    b = np.asarray(b, np.float32)
    Wd = np.asarray(Wd, np.float32)
    bd = np.asarray(bd, np.float32)

    perm_cols = np.concatenate([np.arange(j * P, (j + 1) * P) for j in GATE_PERM])
    Wh_p = Wh[:, perm_cols]                     # (256, 1024)
    Wx_p = Wx[:, perm_cols]                     # (35, 1024)
    b_p = b[perm_cols]                          # (1024,)

    wh_host = np.ascontiguousarray(
        Wh_p.reshape(2, P, G4).transpose(1, 0, 2).reshape(P, 2 * G4)
    ).astype(ml_dtypes.bfloat16)
    # wh layout: [p, k*G4 + col] = Wh_p[k*128 + p, col]
    bvec_host = np.ascontiguousarray(b_p.reshape(8, P).T)  # (128, 8)
    wd_host = np.ascontiguousarray(
        Wd.reshape(2, P, MDN).transpose(1, 0, 2).reshape(P, 2 * MDN)
    ).astype(ml_dtypes.bfloat16)
    bdb_host = np.broadcast_to(bd, (P, MDN)).copy()
    ident_host = np.eye(P, dtype=np.float32).astype(ml_dtypes.bfloat16)

    x = np.concatenate([z_t, a_t], axis=-1)     # (B, t, 35)
    in_maps = []
    for core in range(NCORE):
        xs = x[core * BL:(core + 1) * BL]       # (16, t, 35)
        xTh = np.ascontiguousarray(xs.transpose(2, 1, 0).reshape(IN, t_steps * BL))
        in_maps.append({
            "xT": xTh.astype(ml_dtypes.bfloat16),
            "wh": wh_host,
            "wx": np.ascontiguousarray(Wx_p).astype(ml_dtypes.bfloat16),
            "bvec": bvec_host,
            "wd": wd_host,
            "bdb": bdb_host,
            "ident": ident_host,
        })
    return in_maps


def run(inputs, t_steps=T, trace=False):
    from concourse.bass_utils import run_bass_kernel_spmd
    nc = _get_nc(t_steps)
    in_maps = prep_inputs(**inputs, t_steps=t_steps)
    res = run_bass_kernel_spmd(nc, in_maps, core_ids=list(range(NCORE)), trace=trace)
    return res


def _unscramble(arr, t_steps):
    # device rows are token = t*BL + b; -> (BL, t_steps, d)
    d = arr.shape[-1]
    return np.ascontiguousarray(arr.reshape(t_steps, BL, d).transpose(1, 0, 2))


def assemble(results, t_steps=T):
    logpi = np.concatenate([_unscramble(r["logpi"], t_steps) for r in results], axis=0)
    mu = np.concatenate([_unscramble(r["mu"], t_steps) for r in results], axis=0)
    sigma = np.concatenate([_unscramble(r["sigma"], t_steps) for r in results], axis=0)
    c = np.concatenate([r["cout"] for r in results], axis=0)
    h = np.concatenate([r["hout"] for r in results], axis=0)
    B = logpi.shape[0]
    mu = mu.reshape(B, t_steps, K_MIX, Z)
    sigma = sigma.reshape(B, t_steps, K_MIX, Z)
    return logpi, mu, sigma, (c, h)


def kernel(z_t, a_t, Wx, Wh, b, Wd, bd):
    res = run(dict(z_t=z_t, a_t=a_t, Wx=Wx, Wh=Wh, b=b, Wd=Wd, bd=bd), t_steps=T)
    return assemble(res.results, t_steps=T)


# revision 20
# speedup vs baseline: 1.0599x; 1.0436x over previous
"""MDN-RNN (LSTM + MDN head) Trainium2 kernel, data-parallel over batch on 8 cores.

Shapes: B=128, T=512, Z=32, A=3 -> IN=35, H=256, K=5, MDN=325.
Per-core batch shard Bl=16.

Layout notes (per core):
  - Gate tile order (after host col-permutation of Wx/Wh/b): [i0,i1,f0,f1,o0,o1,g0,g1]
    so gates PSUM (128 part = within-tile gate dim, 8 groups x 16 batch cols):
    i = cols 0:32, f = 32:64, o = 64:96, g = 96:128.
  - xT (35, Bl*T) fp32, token = t*16 + b (step-major).
  - xprojT chunk buffer (128, Tc*128) fp32: col = tl*128 + m*16 + b.
  - hT history (128, 2*(T+1)*16) bf16: col = k*(T+1)*16 + (t+1)*16 + b; slot 0 = h_{-1}=0.
  - head: out (tokens, 325) = hT_tile.T @ Wd (both bf16), tokens on partitions.
"""
import sys
import os

sys.path.insert(0, "/opt/trn_rl_repo")

import numpy as np
import ml_dtypes

import concourse.bass as bass
import concourse.mybir as mybir
import concourse.tile as tile
from concourse import bacc

F32 = mybir.dt.float32
F32R = mybir.dt.float32r
BF16 = mybir.dt.bfloat16
AF = mybir.ActivationFunctionType

P = 128
B_FULL, T, Z, A = 128, 512, 32, 3
IN = Z + A            # 35
H = 256               # hidden
G4 = 4 * H            # 1024
K_MIX = 5
MDN = K_MIX * (1 + 2 * Z)  # 325
NCORE = 8
BL = B_FULL // NCORE  # 16
TC = 64               # chunk length in steps
NCHUNK = T // TC
TOK = BL * T          # 8192 tokens per core
SLOTS = T + 1         # h history slots
HCOL = SLOTS * BL     # columns per k-half in hT

# gate m-tile order: which 128-col slice of the original [i f g o] weight matrix
# lands in psum col-group j. groups: i=0:2, f=2:4, o=4:6, g=6:8
GATE_PERM = [0, 1, 2, 3, 6, 7, 4, 5]


def build(t_steps=T):
    nchunk = t_steps // TC
    nc = bacc.Bacc(None, target_bir_lowering=False)

    xT_d = nc.dram_tensor("xT", [IN, BL * t_steps], BF16, kind="ExternalInput")
    wh_d = nc.dram_tensor("wh", [P, 2 * G4], BF16, kind="ExternalInput")
    wx_d = nc.dram_tensor("wx", [IN, G4], BF16, kind="ExternalInput")
    bvec_d = nc.dram_tensor("bvec", [P, 8], F32, kind="ExternalInput")
    wd_d = nc.dram_tensor("wd", [P, 2 * MDN], BF16, kind="ExternalInput")
    bdb_d = nc.dram_tensor("bdb", [P, MDN], F32, kind="ExternalInput")
    id_d = nc.dram_tensor("ident", [P, P], BF16, kind="ExternalInput")

    logpi_d = nc.dram_tensor("logpi", [BL * t_steps, K_MIX], F32, kind="ExternalOutput")
    mu_d = nc.dram_tensor("mu", [BL * t_steps, K_MIX * Z], F32, kind="ExternalOutput")
    sigma_d = nc.dram_tensor("sigma", [BL * t_steps, K_MIX * Z], F32, kind="ExternalOutput")
    cout_d = nc.dram_tensor("cout", [BL, H], F32, kind="ExternalOutput")
    hout_d = nc.dram_tensor("hout", [BL, H], F32, kind="ExternalOutput")

    hcol = (t_steps + 1) * BL
    ntile_tok = (BL * t_steps) // P  # head token tiles total

    with tile.TileContext(nc) as tc:
        with (
            tc.tile_pool(name="const", bufs=1) as cpool,
            tc.tile_pool(name="state", bufs=1) as spool,
            tc.tile_pool(name="xproj", bufs=2) as xppool,
            tc.tile_pool(name="acts", bufs=3) as apool,
            tc.tile_pool(name="tmp", bufs=3) as tpool,
            tc.tile_pool(name="mdnsb", bufs=3) as mpool,
            tc.tile_pool(name="stage", bufs=1) as stpool,
            tc.tile_pool(name="ps_gates", bufs=2, space="PSUM") as gates_ps,
            tc.tile_pool(name="ps_xp", bufs=2, space="PSUM") as xp_ps,
            tc.tile_pool(name="ps_mdn", bufs=2, space="PSUM") as mdn_ps,
        ):
            # ---- constants / inputs in SBUF ----
            xT = cpool.tile([IN, BL * t_steps], BF16)
            wh = cpool.tile([P, 2 * G4], BF16)
            wx = cpool.tile([IN, G4], BF16)
            bvec = cpool.tile([P, 8], F32)
            wd = cpool.tile([P, 2 * MDN], BF16)
            bdb = cpool.tile([P, MDN], F32)
            ident = cpool.tile([P, P], BF16)
            nc.sync.dma_start(xT[:], xT_d[:])
            nc.sync.dma_start(wh[:], wh_d[:])
            nc.sync.dma_start(wx[:], wx_d[:])
            nc.sync.dma_start(bvec[:], bvec_d[:])
            nc.sync.dma_start(wd[:], wd_d[:])
            nc.sync.dma_start(bdb[:], bdb_d[:])
            nc.sync.dma_start(ident[:], id_d[:])

            # ---- state ----
            hT = spool.tile([P, 2 * hcol], BF16)   # h history, bf16
            cT = spool.tile([P, 2 * BL], F32)      # cell state
            hfin = spool.tile([P, 2 * BL], F32)    # final-step h in fp32
            pi_sb = stpool.tile([P, ntile_tok * K_MIX], F32)
            ls_sb = stpool.tile([P, ntile_tok * K_MIX * Z], F32)

            nc.vector.memset(cT[:], 0.0)
            # zero h_{-1} slots (slot 0 of each k half)
            nc.vector.memset(hT[:, 0:BL], 0.0)
            nc.vector.memset(hT[:, hcol:hcol + BL], 0.0)

            for c in range(nchunk):
                # ---------- xproj phase: chunk c ----------
                xp = xppool.tile([P, TC * P], BF16)  # (128, 8192)
                xp4 = xp[:].rearrange("p (tl m b) -> p tl m b", m=8, b=BL)
                for m in range(8):
                    for nn in range(TC * BL // 512):
                        ps = xp_ps.tile([P, 512], F32)
                        nc.tensor.matmul(
                            ps[:],
                            lhsT=wx[:, m * P:(m + 1) * P],
                            rhs=xT[:, c * TC * BL + nn * 512: c * TC * BL + (nn + 1) * 512],
                            start=True, stop=True,
                        )
                        ps3 = ps[:].rearrange("p (tl b) -> p tl b", b=BL)
                        dest = xp4[:, nn * 32:(nn + 1) * 32, m, :]
                        if nn == 0:
                            nc.vector.tensor_scalar_add(dest, ps3, bvec[:, m:m + 1])
                        else:
                            nc.scalar.activation(dest, ps3, AF.Identity, bias=bvec[:, m:m + 1])

                # ---------- recurrence: chunk c ----------
                for tl in range(TC):
                    t = c * TC + tl
                    psA = gates_ps.tile([P, 96], F32, tag="psA")
                    psB = gates_ps.tile([P, 32], F32, tag="psB")
                    nc.tensor.matmul(
                        psA[:], lhsT=ident[:], rhs=xp[:, tl * P: tl * P + 96],
                        start=True, stop=False,
                    )
                    for m in range(6):
                        for k in range(2):
                            nc.tensor.matmul(
                                psA[:, m * BL:(m + 1) * BL],
                                lhsT=wh[:, k * G4 + m * P: k * G4 + (m + 1) * P],
                                rhs=hT[:, k * hcol + t * BL: k * hcol + (t + 1) * BL],
                                start=False, stop=(m == 5 and k == 1),
                            )
                    nc.tensor.matmul(
                        psB[:], lhsT=ident[:], rhs=xp[:, tl * P + 96: (tl + 1) * P],
                        start=True, stop=False,
                    )
                    for m in range(6, 8):
                        for k in range(2):
                            nc.tensor.matmul(
                                psB[:, (m - 6) * BL:(m - 5) * BL],
                                lhsT=wh[:, k * G4 + m * P: k * G4 + (m + 1) * P],
                                rhs=hT[:, k * hcol + t * BL: k * hcol + (t + 1) * BL],
                                start=False, stop=(m == 7 and k == 1),
                            )
                    act = apool.tile([P, P], F32)
                    nc.scalar.activation(act[:, 0:64], psA[:, 0:64], AF.Sigmoid)
                    nc.scalar.activation(act[:, 96:128], psB[:], AF.Tanh)
                    nc.scalar.activation(act[:, 64:96], psA[:, 64:96], AF.Sigmoid)
                    ig = tpool.tile([P, 2 * BL], F32, tag="ig")
                    fc = tpool.tile([P, 2 * BL], F32, tag="fc")
                    th = tpool.tile([P, 2 * BL], F32, tag="th")
                    nc.vector.tensor_mul(ig[:], act[:, 0:32], act[:, 96:128])
                    nc.vector.tensor_mul(fc[:], act[:, 32:64], cT[:])
                    nc.vector.tensor_add(cT[:], ig[:], fc[:])
                    nc.scalar.activation(th[:], cT[:], AF.Tanh)
                    hdst = hT[:].rearrange("p (k s b) -> p k s b", k=2, b=BL)[:, :, t + 1, :]
                    o2 = act[:, 64:96].rearrange("p (k b) -> p k b", k=2)
                    th2 = th[:].rearrange("p (k b) -> p k b", k=2)
                    nc.vector.tensor_mul(hdst, o2, th2)
                    if t == t_steps - 1:
                        nc.vector.tensor_mul(hfin[:], act[:, 64:96], th[:])

                # ---------- MDN head: chunk c ----------
                for g in range(TC // 8):
                    gi = c * (TC // 8) + g
                    t0 = c * TC + g * 8
                    ps = mdn_ps.tile([P, MDN], F32)
                    for k in range(2):
                        nc.tensor.matmul(
                            ps[:],
                            lhsT=hT[:, k * hcol + (t0 + 1) * BL: k * hcol + (t0 + 9) * BL],
                            rhs=wd[:, k * MDN:(k + 1) * MDN],
                            start=(k == 0), stop=(k == 1),
                        )
                    sb = mpool.tile([P, MDN], F32)
                    nc.vector.tensor_add(sb[:], ps[:], bdb[:])
                    # mu straight out
                    nc.sync.dma_start(
                        mu_d[gi * P:(gi + 1) * P, :],
                        sb[:, K_MIX:K_MIX + K_MIX * Z],
                    )
                    # stage pi and log_sigma
                    nc.vector.tensor_copy(pi_sb[:, gi * K_MIX:(gi + 1) * K_MIX], sb[:, 0:K_MIX])
                    nc.vector.tensor_copy(
                        ls_sb[:, gi * K_MIX * Z:(gi + 1) * K_MIX * Z],
                        sb[:, K_MIX + K_MIX * Z:],
                    )

            # ---------- postproc ----------
            # sigma = exp(ls) + 1e-6
            nc.scalar.activation(ls_sb[:], ls_sb[:], AF.Exp)
            nc.vector.tensor_scalar_add(ls_sb[:], ls_sb[:], 1e-6)
            # log_pi = pi - ln(sum(exp(pi), axis=group))
            expv = stpool.tile([P, ntile_tok * K_MIX], F32)
            sums = stpool.tile([P, ntile_tok], F32)
            lns = stpool.tile([P, ntile_tok], F32)
            nc.scalar.activation(expv[:], pi_sb[:], AF.Exp)
            nc.vector.tensor_reduce(
                sums[:].unsqueeze(2),
                expv[:].rearrange("p (g j) -> p g j", j=K_MIX),
                axis=mybir.AxisListType.X, op=mybir.AluOpType.add,
            )
            nc.scalar.activation(lns[:], sums[:], AF.Ln)
            lpi = stpool.tile([P, ntile_tok * K_MIX], F32)
            nc.vector.tensor_tensor(
                lpi[:].rearrange("p (g j) -> p g j", j=K_MIX),
                pi_sb[:].rearrange("p (g j) -> p g j", j=K_MIX),
                lns[:].unsqueeze(2).to_broadcast([P, ntile_tok, K_MIX]),
                mybir.AluOpType.subtract,
            )
            # DMA outs per token tile
            for gi in range(ntile_tok):
                t0 = gi * 8
                nc.sync.dma_start(
                    logpi_d[gi * P:(gi + 1) * P, :],
                    lpi[:, gi * K_MIX:(gi + 1) * K_MIX],
                )
                nc.sync.dma_start(
                    sigma_d[gi * P:(gi + 1) * P, :],
                    ls_sb[:, gi * K_MIX * Z:(gi + 1) * K_MIX * Z],
                )

            # final carry
            for k in range(2):
                nc.sync.dma_start(
                    cout_d[:, k * P:(k + 1) * P].transpose([1, 0]),
                    cT[:, k * BL:(k + 1) * BL],
                )
                nc.sync.dma_start(
                    hout_d[:, k * P:(k + 1) * P].transpose([1, 0]),
                    hfin[:, k * BL:(k + 1) * BL],
                )

    nc.finalize()
    return nc


# ---------------- host side ----------------
_cached = {}


def _get_nc(t_steps):
    if t_steps not in _cached:
        _cached[t_steps] = build(t_steps)
    return _cached[t_steps]


def prep_inputs(z_t, a_t, Wx, Wh, b, Wd, bd, t_steps=T):
    """Build per-core input maps."""
    z_t = np.asarray(z_t, np.float32)[:, :t_steps]
    a_t = np.asarray(a_t, np.float32)[:, :t_steps]
    Wx = np.asarray(Wx, np.float32)
    Wh = np.asarray(Wh, np.float32)
    b = np.asarray(b, np.float32)
    Wd = np.asarray(Wd, np.float32)
    bd = np.asarray(bd, np.float32)

    perm_cols = np.concatenate([np.arange(j * P, (j + 1) * P) for j in GATE_PERM])
    Wh_p = Wh[:, perm_cols]                     # (256, 1024)
    Wx_p = Wx[:, perm_cols]                     # (35, 1024)
    b_p = b[perm_cols]                          # (1024,)

    wh_host = np.ascontiguousarray(
        Wh_p.reshape(2, P, G4).transpose(1, 0, 2).reshape(P, 2 * G4)
    ).astype(ml_dtypes.bfloat16)
    # wh layout: [p, k*G4 + col] = Wh_p[k*128 + p, col]
    bvec_host = np.ascontiguousarray(b_p.reshape(8, P).T)  # (128, 8)
    wd_host = np.ascontiguousarray(
        Wd.reshape(2, P, MDN).transpose(1, 0, 2).reshape(P, 2 * MDN)
    ).astype(ml_dtypes.bfloat16)
    bdb_host = np.broadcast_to(bd, (P, MDN)).copy()
    ident_host = np.eye(P, dtype=np.float32).astype(ml_dtypes.bfloat16)

    x = np.concatenate([z_t, a_t], axis=-1)     # (B, t, 35)
    in_maps = []
    for core in range(NCORE):
        xs = x[core * BL:(core + 1) * BL]       # (16, t, 35)
        xTh = np.ascontiguousarray(xs.transpose(2, 1, 0).reshape(IN, t_steps * BL))
        in_maps.append({
            "xT": xTh.astype(ml_dtypes.bfloat16),
            "wh": wh_host,
            "wx": np.ascontiguousarray(Wx_p).astype(ml_dtypes.bfloat16),
            "bvec": bvec_host,
            "wd": wd_host,
            "bdb": bdb_host,
            "ident": ident_host,
        })
    return in_maps


def run(inputs, t_steps=T, trace=False):
    from concourse.bass_utils import run_bass_kernel_spmd
    nc = _get_nc(t_steps)
    in_maps = prep_inputs(**inputs, t_steps=t_steps)
    res = run_bass_kernel_spmd(nc, in_maps, core_ids=list(range(NCORE)), trace=trace)
    return res


def _unscramble(arr, t_steps):
    # device rows are token = t*BL + b; -> (BL, t_steps, d)
    d = arr.shape[-1]
    return np.ascontiguousarray(arr.reshape(t_steps, BL, d).transpose(1, 0, 2))


def assemble(results, t_steps=T):
    logpi = np.concatenate([_unscramble(r["logpi"], t_steps) for r in results], axis=0)
    mu = np.concatenate([_unscramble(r["mu"], t_steps) for r in results], axis=0)
    sigma = np.concatenate([_unscramble(r["sigma"], t_steps) for r in results], axis=0)
    c = np.concatenate([r["cout"] for r in results], axis=0)
    h = np.concatenate([r["hout"] for r in results], axis=0)
    B = logpi.shape[0]
    mu = mu.reshape(B, t_steps, K_MIX, Z)
    sigma = sigma.reshape(B, t_steps, K_MIX, Z)
    return logpi, mu, sigma, (c, h)


def kernel(z_t, a_t, Wx, Wh, b, Wd, bd):
    res = run(dict(z_t=z_t, a_t=a_t, Wx=Wx, Wh=Wh, b=b, Wd=Wd, bd=bd), t_steps=T)
    return assemble(res.results, t_steps=T)
